# revision 4
# baseline (speedup 1.0000x reference)
"""DeconvCapsuleLayer Trainium2 kernel.

Strategy: data-parallel over batch (B=8 -> 1 image per NeuronCore).
Per core:
  - deconv (conv_transpose stride2 k4 SAME) computed as 4 sub-pixel phases;
    each phase = 4 taps of K=32 matmuls accumulated in PSUM (W stationary,
    out = [64(oc,oa), pixels]).
  - PE transpose to pixel-major [pixels, (ic,oc,oa)].
  - dynamic routing (3 iters) on DVE/ACT in pixel-major layout with free-dim
    broadcasts only.
Host: pads/transposes inputs, reassembles phase images.
"""

import os
import sys
from contextlib import ExitStack

import numpy as np

for _p in ("/opt/trn_rl_repo", os.path.expanduser("~/.axon_site/_ro/trn_rl_repo")):
    if os.path.isdir(_p) and _p not in sys.path:
        sys.path.insert(0, _p)

import concourse.bass as bass
import concourse.bacc as bacc
import concourse.tile as tile
from concourse import mybir
from concourse.bass_utils import run_bass_kernel_spmd

F32 = mybir.dt.float32
AX = mybir.AxisListType
OP = mybir.AluOpType
AF = mybir.ActivationFunctionType

B, H, Wd, IC, IA = 8, 56, 56, 8, 32
OC, OA = 4, 16
PH, PW = 58, 58  # padded input spatial
NPIX = 56 * 56   # pixels per phase image
# tap tables: KH[parity] = kernel taps, DH[parity] = input shifts
KH = {0: [1, 3], 1: [0, 2]}
DH = {0: [0, -1], 1: [1, 0]}

_CACHE = {}


def _squash_tiles(nc, pool, t_ap, out_ap, tag):
    """out = t * sqrt(nsq)/(1+nsq), nsq = sum_oa t^2  (t: [112, 64])."""
    sq = pool.tile([112, 64], F32, tag="mid")
    nc.vector.tensor_mul(sq[:], t_ap, t_ap)
    nsq = pool.tile([112, 4], F32, tag="sml")
    nc.vector.tensor_reduce(
        nsq[:], sq[:].rearrange("p (oc oa) -> p oc oa", oc=4), axis=AX.X, op=OP.add
    )
    s = pool.tile([112, 4], F32, tag="sml")
    nc.scalar.sqrt(s[:], nsq[:])
    u = pool.tile([112, 4], F32, tag="sml")
    nc.vector.tensor_scalar_add(u[:], nsq[:], 1.0)
    rc = pool.tile([112, 4], F32, tag="sml")
    nc.vector.reciprocal(rc[:], u[:])
    f = pool.tile([112, 4], F32, tag="sml")
    nc.vector.tensor_mul(f[:], s[:], rc[:])
    f_bc = f[:].unsqueeze(2).broadcast_to([112, 4, 16])
    t3 = t_ap.rearrange("p (oc oa) -> p oc oa", oc=4)
    nc.vector.tensor_mul(out_ap.rearrange("p (oc oa) -> p oc oa", oc=4), t3, f_bc)


def _build_nc():
    if "nc" in _CACHE:
        return _CACHE["nc"]
    nc = bacc.Bacc("TRN2", target_bir_lowering=False, debug=False)
    x_d = nc.dram_tensor("x", [32, IC * PH * PW], F32, kind="ExternalInput")
    wt_d = nc.dram_tensor("wt", [32, 1024], F32, kind="ExternalInput")
    cst_d = nc.dram_tensor("cst", [128, 128], F32, kind="ExternalInput")
    out_d = nc.dram_tensor("out", [4, NPIX, 64], F32, kind="ExternalOutput")

    with tile.TileContext(nc) as tc, ExitStack() as ctx:
        cpool = ctx.enter_context(tc.tile_pool(name="const", bufs=1))
        xwpool = ctx.enter_context(tc.tile_pool(name="xw", bufs=2))
        wt_sb = cpool.tile([32, 1024], F32, tag="wt")
        nc.sync.dma_start(wt_sb[:], wt_d.ap())
        cst_sb = cpool.tile([128, 128], F32, tag="cst")
        nc.sync.dma_start(cst_sb[:], cst_d.ap())
        bias_ap = cst_sb[0:112, 0:64]
        ident = cst_sb[0:64, 64:128]

        vpool = ctx.enter_context(tc.tile_pool(name="votes", bufs=2))
        pmpool = ctx.enter_context(tc.tile_pool(name="pm", bufs=2))
        pspool = ctx.enter_context(tc.tile_pool(name="ps", bufs=2, space="PSUM"))
        tppool = ctx.enter_context(tc.tile_pool(name="tp", bufs=2, space="PSUM"))
        rt = ctx.enter_context(tc.tile_pool(name="rt", bufs=10))
        opool = ctx.enter_context(tc.tile_pool(name="outp", bufs=3))

        x_dv = x_d.ap().rearrange("k (ic h w) -> k ic h w", ic=IC, h=PH, w=PW)

        for p in range(4):
            ph, pw = p >> 1, p & 1
            for mb in range(7):
                xw = xwpool.tile([32, IC * 10 * PW], F32, tag="xw")
                nc.sync.dma_start(
                    xw[:].rearrange("k (ic h w) -> k ic h w", ic=IC, h=10, w=PW),
                    x_dv[:, :, mb * 8 : mb * 8 + 10, :],
                )
                x_v = xw[:].rearrange("k (ic h w) -> k ic h w", ic=IC, h=10, w=PW)
                votes_sb = vpool.tile([64, 8 * 448], F32, tag="vsb")
                for ic in range(IC):
                    ps = pspool.tile([64, 448], F32, tag="ps")
                    for j in range(4):
                        jh, jw = j >> 1, j & 1
                        dh = DH[ph][jh]
                        dw = DH[pw][jw]
                        rhs = x_v[
                            :, ic, 1 + dh : 1 + dh + 8, 1 + dw : 1 + dw + 56
                        ]
                        nc.tensor.matmul(
                            ps[:],
                            wt_sb[:, (p * 4 + j) * 64 : (p * 4 + j + 1) * 64],
                            rhs,
                            start=(j == 0),
                            stop=(j == 3),
                        )
                    nc.scalar.copy(votes_sb[:, ic * 448 : (ic + 1) * 448], ps[:])

                for q in range(4):
                    tp = tppool.tile([112, 512], F32, tag="tp")
                    for ic in range(IC):
                        nc.tensor.transpose(
                            tp[:, ic * 64 : (ic + 1) * 64],
                            votes_sb[:, ic * 448 + q * 112 : ic * 448 + (q + 1) * 112],
                            ident,
                        )
                    v = pmpool.tile([112, 512], F32, tag="v")
                    nc.scalar.copy(v[:], tp[:])

                    # ---- routing on v [112, (ic,oc,oa)] ----
                    v4 = v[:].rearrange("p (ic oc oa) -> p ic oc oa", ic=8, oc=4)
                    v_jic = v[:].rearrange("p (ic j) -> p j ic", ic=8)

                    # iter 1: r uniform 0.25
                    Sv = rt.tile([112, 64], F32, tag="mid")
                    nc.vector.tensor_reduce(Sv[:], v_jic, axis=AX.X, op=OP.add)
                    t1 = rt.tile([112, 64], F32, tag="mid")
                    nc.vector.scalar_tensor_tensor(
                        t1[:], Sv[:], 0.25, bias_ap, op0=OP.mult, op1=OP.add
                    )
                    act1 = rt.tile([112, 64], F32, tag="actA")
                    _squash_tiles(nc, rt, t1[:], act1[:], "a")

                    dl = rt.tile([112, 32], F32, tag="dlg")
                    act_prev = act1
                    for it in (2, 3):
                        tmp = rt.tile([112, 512], F32, tag="big")
                        a_bc = (
                            act_prev[:]
                            .rearrange("p (oc oa) -> p oc oa", oc=4)
                            .unsqueeze(1)
                            .broadcast_to([112, 8, 4, 16])
                        )
                        tmp4 = tmp[:].rearrange(
                            "p (ic oc oa) -> p ic oc oa", ic=8, oc=4
                        )
                        nc.vector.tensor_mul(tmp4, v4, a_bc)
                        if it == 2:
                            nc.vector.tensor_reduce(
                                dl[:],
                                tmp[:].rearrange("p (g oa) -> p g oa", g=32),
                                axis=AX.X,
                                op=OP.add,
                            )
                        else:
                            dlb = rt.tile([112, 32], F32, tag="mid")
                            nc.vector.tensor_reduce(
                                dlb[:],
                                tmp[:].rearrange("p (g oa) -> p g oa", g=32),
                                axis=AX.X,
                                op=OP.add,
                            )
                            nc.vector.tensor_add(dl[:], dl[:], dlb[:])
                        # softmax over oc (no max-sub; logits are small)
                        e = rt.tile([112, 32], F32, tag="mid")
                        nc.scalar.activation(e[:], dl[:], AF.Exp)
                        se = rt.tile([112, 8], F32, tag="sml")
                        nc.vector.tensor_reduce(
                            se[:],
                            e[:].rearrange("p (ic oc) -> p ic oc", oc=4),
                            axis=AX.X,
                            op=OP.add,
                        )
                        rcp = rt.tile([112, 8], F32, tag="sml")
                        nc.vector.reciprocal(rcp[:], se[:])
                        r = rt.tile([112, 32], F32, tag="mid")
                        nc.vector.tensor_mul(
                            r[:].rearrange("p (ic oc) -> p ic oc", oc=4),
                            e[:].rearrange("p (ic oc) -> p ic oc", oc=4),
                            rcp[:].unsqueeze(2).broadcast_to([112, 8, 4]),
                        )
                        # preact = sum_ic r*v + b
                        rv = rt.tile([112, 512], F32, tag="big")
                        r_bc = (
                            r[:]
                            .rearrange("p (ic oc) -> p ic oc", oc=4)
                            .unsqueeze(3)
                            .broadcast_to([112, 8, 4, 16])
                        )
                        nc.vector.tensor_mul(
                            rv[:].rearrange("p (ic oc oa) -> p ic oc oa", ic=8, oc=4),
                            v4,
                            r_bc,
                        )
                        pre = rt.tile([112, 64], F32, tag="mid")
                        nc.vector.tensor_reduce(
                            pre[:],
                            rv[:].rearrange("p (ic j) -> p j ic", ic=8),
                            axis=AX.X,
                            op=OP.add,
                        )
                        tb = rt.tile([112, 64], F32, tag="mid")
                        nc.vector.tensor_add(tb[:], pre[:], bias_ap)
                        if it == 2:
                            act2 = rt.tile([112, 64], F32, tag="actA")
                            _squash_tiles(nc, rt, tb[:], act2[:], "b")
                            act_prev = act2
                        else:
                            act3 = opool.tile([112, 64], F32, tag="act3")
                            _squash_tiles(nc, rt, tb[:], act3[:], "c")
                            base = mb * 448 + q * 112
                            nc.sync.dma_start(
                                out_d.ap()[p, base : base + 112, :], act3[:]
                            )
    nc.compile()
    _CACHE["nc"] = nc
    return nc


def _prep_inputs(input_tensor, W, b):
    x = np.ascontiguousarray(np.asarray(input_tensor, np.float32))
    Wk = np.asarray(W, np.float32)
    bb = np.asarray(b, np.float32).reshape(OC, OA)
    xpad = np.zeros((B, IA, IC, PH, PW), np.float32)
    xpad[:, :, :, 1:57, 1:57] = x.transpose(0, 4, 3, 1, 2)
    wt = np.zeros((32, 1024), np.float32)
    for p in range(4):
        ph, pw = p >> 1, p & 1
        for j in range(4):
            jh, jw = j >> 1, j & 1
            kh, kw = KH[ph][jh], KH[pw][jw]
            wt[:, (p * 4 + j) * 64 : (p * 4 + j + 1) * 64] = Wk[kh, kw].T
    cst = np.zeros((128, 128), np.float32)
    cst[:, :64] = bb.reshape(1, 64)
    cst[:64, 64:128] = np.eye(64, dtype=np.float32)
    in_maps = [
        {"x": np.ascontiguousarray(xpad[bi].reshape(32, -1)), "wt": wt, "cst": cst}
        for bi in range(B)
    ]
    return in_maps


def _unshard(results):
    outs = []
    for bi in range(B):
        o = np.asarray(results[bi]["out"], np.float32)
        o = o.reshape(2, 2, 56, 56, OC, OA).transpose(2, 0, 3, 1, 4, 5)
        outs.append(o.reshape(112, 112, OC, OA))
    return np.stack(outs)


def kernel(input_tensor, W, b):
    nc = _build_nc()
    in_maps = _prep_inputs(input_tensor, W, b)
    res = run_bass_kernel_spmd(nc, in_maps, core_ids=list(range(8)))
    return _unshard(res.results)
